# revision 20
# baseline (speedup 1.0000x reference)
"""Causal squeeze-excite 1d on 8 TRN2 NeuronCores.

Reference computation (per batch b):
    y = causal_ema(x)                      # y[t] = (1-a) y[t-1] + a x[t], y[0] = x[0]
    h = relu(w1 @ y[:, t] + b1)            # (32,)  per time step
    g = sigmoid(w2 @ h + b2)               # (512,) per time step
    out[:, t] = x[:, t] * g
Sharding: data-parallel over batch; core i gets x[2i:2i+2].

Structure (v4, fp16 IO):
  - x/out/weights travel as fp16: halves HBM traffic (the kernel is
    DMA-bound at ~358 GB/s/core); fp16's 2^-11 rounding is far inside
    tolerance.  Host lays DRAM out per (b, chunk) so every load is 128
    descriptors x one contiguous run (<= 8 KB).
  - EMA commutes with the channel projection: w1 @ ema(x) == ema((a*w1) @ x),
    so the DVE scan runs on a 32-row projected sequence, not [512, T].
  - Both batches stack in PSUM partitions (b0 rows 0-31, b1 rows 32-63 via
    PE tile placement), so ONE scan / ONE relu covers both batches.
    b0/b1 matmul chains are emission-interleaved so the two PE tiles
    co-execute (~2x PE throughput, robust to HAM K=4/8 throttling).
  - b1 rides the DVE relu (fused add+max); b2 rides the sigmoid
    ACTIVATE's per-partition bias.  ACT runs sigmoids only -- it is the
    busiest compute engine, so everything else is kept off it.
  - All loads issue up front on the Sync HWDGE ring; stores go out on
    the GpSimd SWDGE ring so a store trigger waiting on a sigmoid can
    never head-of-line block the load stream.  Chunk schedule
    1024/1024/1024/768/256 minimizes ACTIVATE instruction overhead
    while keeping the serial tail (last chunk) short.
"""

import numpy as np
from contextlib import ExitStack

import concourse.bass as bass
import concourse.bacc as bacc
import concourse.tile as tile
import concourse.mybir as mybir
from concourse.bass_utils import run_bass_kernel_spmd

F32 = mybir.dt.float32
F16 = mybir.dt.float16

N_CORES = 8
B, C, T = 16, 512, 4096
CSQ = 32          # squeeze dim
P = 128           # SBUF partitions
NCB = C // P      # channel blocks (4)
B_LOC = B // N_CORES          # batches per core (2)
M2 = B_LOC * CSQ  # stacked mm1 output rows (64)
TS = 512          # max matmul / scan sub-tile (one PSUM bank)
CHUNKS = [(0, 1024), (1024, 1024), (2048, 1024), (3072, 768), (3840, 256)]
NTH = len(CHUNKS)
TCMAX = max(c[1] for c in CHUNKS)


def _subtiles(tcc):
    """Split a chunk into <=TS sub-tiles."""
    out, o = [], 0
    while o < tcc:
        s = min(TS, tcc - o)
        out.append((o, s))
        o += s
    return out


def build_nc(B_loc, cw, C_=C, T_=T):
    assert B_loc == B_LOC
    d = 1.0 - 1.0 / cw
    assert sum(c[1] for c in CHUNKS) == T_

    nc = bacc.Bacc(trn_type="TRN2")
    # x/out DRAM layout: flat [p, b*T*NCB] with per-(b, chunk) contiguous
    # blocks laid [cb, t] (see make_in_maps).
    xin = nc.declare_dram_parameter("x", [P, B_loc * NCB * T_], F16,
                                    isOutput=False)
    w1e = nc.declare_dram_parameter("w1e", [P, NCB * CSQ], F16, isOutput=False)
    w2d = nc.declare_dram_parameter("w2d", [M2, C_], F16, isOutput=False)
    b1d = nc.declare_dram_parameter("b1d", [M2, 1], F32, isOutput=False)
    b2e = nc.declare_dram_parameter("b2e", [P, NCB], F32, isOutput=False)
    out = nc.declare_dram_parameter("out", [P, B_loc * NCB * T_], F16,
                                    isOutput=True)

    def dslice(dram, b, ci):
        t0, tcc = CHUNKS[ci]
        off = b * (NCB * T_) + t0 * NCB
        return dram[:, off:off + NCB * tcc]

    with ExitStack() as ctx:
        tc = ctx.enter_context(tile.TileContext(nc))
        const = ctx.enter_context(tc.tile_pool(name="const", bufs=1))
        # All chunks live in SBUF at once: loads all issue up front.
        xpool = ctx.enter_context(
            tc.tile_pool(name="xp", bufs=B_loc * NTH))
        opool = ctx.enter_context(tc.tile_pool(name="op", bufs=4))
        gpool = ctx.enter_context(tc.tile_pool(name="gp", bufs=4))
        upool = ctx.enter_context(tc.tile_pool(name="up", bufs=3))
        hpool = ctx.enter_context(tc.tile_pool(name="hp", bufs=3))
        cpool = ctx.enter_context(tc.tile_pool(name="cp", bufs=2))
        php = ctx.enter_context(tc.tile_pool(name="php", bufs=2, space="PSUM"))
        pgp = ctx.enter_context(tc.tile_pool(name="pgp", bufs=3, space="PSUM"))

        # Consts ride the Scalar HWDGE ring so the Sync ring starts on x
        # immediately (they finish long before the first sigmoid).
        w1_t = const.tile([P, NCB * CSQ], F16, tag="w1e")
        nc.scalar.dma_start(w1_t[:], w1e[:])
        w2_t = const.tile([M2, C_], F16, tag="w2d")
        nc.scalar.dma_start(w2_t[:], w2d[:])
        b1_t = const.tile([M2, 1], F32, tag="b1d")
        nc.scalar.dma_start(b1_t[:], b1d[:])
        b2_t = const.tile([P, NCB], F32, tag="b2e")
        nc.scalar.dma_start(b2_t[:], b2e[:])
        dconst = const.tile([M2, TS], F32, tag="dconst")
        nc.vector.memset(dconst[:], d)

        # Issue every load immediately; the Sync ring drains them at HBM
        # rate with nothing in the way.
        xts = {}
        for ci in range(NTH):
            _, tcc = CHUNKS[ci]
            for b in range(B_loc):
                xt = xpool.tile([P, NCB * TCMAX], F16, tag="x",
                                name=f"x{b}_{ci}")
                xw3 = xt[:].rearrange("p (cb t) -> p cb t", cb=NCB, t=TCMAX)
                xv3 = dslice(xin, b, ci).rearrange(
                    "p (cb t) -> p cb t", cb=NCB, t=tcc)
                nc.sync.dma_start(xw3[:, :, 0:tcc], xv3)
                xts[(b, ci)] = xt

        ph_pre = {}

        def phase1(ci):
            # mm1 for chunk ci, both batches stacked into one PSUM tile
            # (b0 -> rows 0-31, b1 -> rows 32-63): the two accumulation
            # chains are emission-interleaved so PE col-tiles (0,0) and
            # (0,32) co-execute.
            _, tcc = CHUNKS[ci]
            stl = _subtiles(tcc)
            xws_ = [xts[(b, ci)][:].rearrange("p (cb t) -> p cb t", cb=NCB,
                                              t=TCMAX)
                    for b in range(B_loc)]
            phs = []
            for o, s in stl:
                ph = php.tile([M2, TS], F32, tag="ph")
                for cb in range(NCB):
                    for b in range(B_loc):
                        nc.tensor.matmul(
                            ph[b * CSQ:(b + 1) * CSQ, 0:s],
                            w1_t[:, cb * CSQ:(cb + 1) * CSQ],
                            xws_[b][:, cb, o:o + s],
                            start=(cb == 0), stop=(cb == NCB - 1))
                phs.append(ph)
            ph_pre[ci] = phs

        phase1(0)
        carry = None
        for th in range(NTH):
            if th + 1 < NTH:
                phase1(th + 1)
            t0, tcc = CHUNKS[th]
            stl = _subtiles(tcc)
            phs = ph_pre.pop(th)
            # Phase 2: one scan per sub-tile + one fused relu per chunk,
            # covering BOTH batches (stacked rows).
            ut = upool.tile([M2, TCMAX], F32, tag="u")
            for k, (o, s) in enumerate(stl):
                if th == 0 and k == 0:
                    # u_0 = cw * p_0 makes y[0] = x[0] exact.
                    init = cpool.tile([M2, 1], F32, tag="c")
                    nc.vector.tensor_scalar_mul(
                        init[:], phs[k][:, 0:1], float(cw))
                    init_ap = init[:]
                else:
                    init_ap = carry
                nc.vector.tensor_tensor_scan(
                    ut[:, o:o + s], dconst[:, 0:s],
                    phs[k][:, 0:s], init_ap,
                    mybir.AluOpType.mult, mybir.AluOpType.add)
                carry = ut[:, o + s - 1:o + s]
            # Fused (u + b1) -> max(., 0) on the DVE keeps ACT free for
            # sigmoids.  Bias row layout matches the stacked batches.
            ht = hpool.tile([M2, TCMAX], F16, tag="h")
            nc.vector.tensor_scalar(
                ht[:, 0:tcc], ut[:, 0:tcc], b1_t[:], 0.0,
                mybir.AluOpType.add, mybir.AluOpType.max)
            # Phase 3: mm2 + sigmoid per (b, cb); all sub-tiles of the
            # chunk land in one PSUM tile -> one sigmoid each, b2 riding
            # the ACTIVATE bias.  b0/b1 interleaved (PE row-tiles T0/T4).
            gts = [gpool.tile([P, NCB * TCMAX], F16, tag="g", name=f"g{b}")
                   for b in range(B_loc)]
            gws = [g[:].rearrange("p (cb t) -> p cb t", cb=NCB, t=TCMAX)
                   for g in gts]
            for cb in range(NCB):
                pgs = [pgp.tile([P, TCMAX], F32, tag="pg", name=f"pg{b}")
                       for b in range(B_loc)]
                for o, s in stl:
                    for b in range(B_loc):
                        nc.tensor.matmul(
                            pgs[b][:, o:o + s],
                            w2_t[b * CSQ:(b + 1) * CSQ, cb * P:(cb + 1) * P],
                            ht[b * CSQ:(b + 1) * CSQ, o:o + s],
                            start=True, stop=True)
                for b in range(B_loc):
                    nc.scalar.activation(
                        gws[b][:, cb, 0:tcc], pgs[b][:, 0:tcc],
                        mybir.ActivationFunctionType.Sigmoid,
                        bias=b2_t[:, cb:cb + 1])
            # Phase 4: one gate multiply + one store per batch.  The
            # multiply writes a fresh fp16 tile (all-16-bit, packed DVE
            # rate); the store's output block is [cb, t]-contiguous in
            # DRAM and goes out on the GpSimd SWDGE ring.
            for b in range(B_loc):
                ot = opool.tile([P, NCB * TCMAX], F16, tag="o", name=f"o{b}")
                ow = ot[:].rearrange("p (cb t) -> p cb t", cb=NCB, t=TCMAX)
                xw = xts.pop((b, th))[:].rearrange(
                    "p (cb t) -> p cb t", cb=NCB, t=TCMAX)
                nc.vector.tensor_mul(
                    ow[:, :, 0:tcc], xw[:, :, 0:tcc], gws[b][:, :, 0:tcc])
                dv = dslice(out, b, th).rearrange(
                    "p (cb t) -> p cb t", cb=NCB, t=tcc)
                nc.gpsimd.dma_start(dv, ow[:, :, 0:tcc])
    nc.compile()
    return nc


def make_in_maps(x, w1, b1, w2, b2, cw, n_cores=N_CORES):
    """Host-side shard + weight prep. Returns per-core input maps."""
    a = 1.0 / cw
    C_ = w2.shape[0]
    b_loc = x.shape[0] // n_cores

    w1sT = (np.asarray(w1) * a).T.astype(np.float32)      # [C, CSQ]
    w1e = np.empty((P, NCB * CSQ), dtype=np.float16)
    for cb in range(NCB):
        w1e[:, cb * CSQ:(cb + 1) * CSQ] = w1sT[cb * P:(cb + 1) * P, :]

    w2d = np.empty((M2, C_), dtype=np.float16)
    for b in range(b_loc):
        w2d[b * CSQ:(b + 1) * CSQ, :] = np.asarray(w2).T

    b1d = np.empty((M2, 1), dtype=np.float32)
    for b in range(b_loc):
        b1d[b * CSQ:(b + 1) * CSQ, 0] = np.asarray(b1)

    b2e = np.asarray(b2).astype(np.float32).reshape(NCB, P).T.copy()

    # [B, C, T] -> per-core flat [P, b*(chunk-major [cb, t])] fp16.
    x16 = np.asarray(x).astype(np.float16)
    x16 = x16.reshape(n_cores, b_loc, NCB, P, T)
    xf = np.empty((n_cores, P, b_loc * NCB * T), dtype=np.float16)
    for b in range(b_loc):
        for (t0, tcc) in CHUNKS:
            off = b * (NCB * T) + t0 * NCB
            blk = x16[:, b, :, :, t0:t0 + tcc]        # [core, cb, p, t]
            xf[:, :, off:off + NCB * tcc] = (
                blk.transpose(0, 2, 1, 3).reshape(n_cores, P, NCB * tcc))

    return [
        {"x": xf[i], "w1e": w1e, "w2d": w2d, "b1d": b1d, "b2e": b2e}
        for i in range(n_cores)
    ]


def unshard_out(results, n_cores=N_CORES, b_loc=B_LOC):
    """Per-core flat fp16 -> full [B, C, T] fp32."""
    o = np.stack([r["out"] for r in results], axis=0)  # [core, P, b*NCB*T]
    full = np.empty((n_cores, b_loc, NCB, P, T), dtype=np.float32)
    for b in range(b_loc):
        for (t0, tcc) in CHUNKS:
            off = b * (NCB * T) + t0 * NCB
            blk = o[:, :, off:off + NCB * tcc].reshape(n_cores, P, NCB, tcc)
            full[:, b, :, :, t0:t0 + tcc] = blk.transpose(0, 2, 1, 3)
    return full.reshape(B, C, T)


_NC_CACHE = {}


def kernel(x, w1, b1, w2, b2, context_window):
    cw = int(context_window)
    x = np.asarray(x)
    key = (cw, x.shape)
    if key not in _NC_CACHE:
        _NC_CACHE[key] = build_nc(x.shape[0] // N_CORES, cw)
    nc = _NC_CACHE[key]
    in_maps = make_in_maps(
        np.asarray(x), np.asarray(w1), np.asarray(b1),
        np.asarray(w2), np.asarray(b2), cw)
    res = run_bass_kernel_spmd(nc, in_maps, core_ids=list(range(N_CORES)))
    return unshard_out(res.results)


# revision 25
# speedup vs baseline: 1.0233x; 1.0233x over previous
"""Causal squeeze-excite 1d on 8 TRN2 NeuronCores.

Reference computation (per batch b):
    y = causal_ema(x)                      # y[t] = (1-a) y[t-1] + a x[t], y[0] = x[0]
    h = relu(w1 @ y[:, t] + b1)            # (32,)  per time step
    g = sigmoid(w2 @ h + b2)               # (512,) per time step
    out[:, t] = x[:, t] * g
Sharding: data-parallel over batch; core i gets x[2i:2i+2].

Structure (v4, fp16 IO):
  - x/out/weights travel as fp16: halves HBM traffic (the kernel is
    DMA-bound at ~358 GB/s/core); fp16's 2^-11 rounding is far inside
    tolerance.  Host lays DRAM out per (b, chunk) so every load is 128
    descriptors x one contiguous run (<= 8 KB).
  - EMA commutes with the channel projection: w1 @ ema(x) == ema((a*w1) @ x),
    so the DVE scan runs on a 32-row projected sequence, not [512, T].
  - Both batches stack in PSUM partitions (b0 rows 0-31, b1 rows 32-63 via
    PE tile placement), so ONE scan / ONE relu covers both batches.
    b0/b1 matmul chains are emission-interleaved so the two PE tiles
    co-execute (~2x PE throughput, robust to HAM K=4/8 throttling).
  - b1 rides the DVE relu (fused add+max); b2 rides the sigmoid
    ACTIVATE's per-partition bias.  ACT runs sigmoids only -- it is the
    busiest compute engine, so everything else is kept off it.
  - All loads issue up front on the Sync HWDGE ring; stores go out on
    the GpSimd SWDGE ring so a store trigger waiting on a sigmoid can
    never head-of-line block the load stream.  Chunk schedule
    1024/1024/1024/768/256 minimizes ACTIVATE instruction overhead
    while keeping the serial tail (last chunk) short.
"""

import numpy as np
from contextlib import ExitStack

import concourse.bass as bass
import concourse.bacc as bacc
import concourse.tile as tile
import concourse.mybir as mybir
from concourse.bass_utils import run_bass_kernel_spmd

F32 = mybir.dt.float32
F16 = mybir.dt.float16

N_CORES = 8
B, C, T = 16, 512, 4096
CSQ = 32          # squeeze dim
P = 128           # SBUF partitions
NCB = C // P      # channel blocks (4)
B_LOC = B // N_CORES          # batches per core (2)
M2 = B_LOC * CSQ  # stacked mm1 output rows (64)
TS = 512          # max matmul / scan sub-tile (one PSUM bank)
# Few, large chunks: each ACTIVATE (sigmoid) costs (N+352)/1.2 ns plus
# ~2 semaphore ops, so ACT time is minimized by the fewest chunks whose
# pg tiles still fit PSUM (pgp 2x3 banks + php 2x1 = 8 banks).
CHUNKS = [(0, 1024), (1024, 1536), (2560, 1536)]
NTH = len(CHUNKS)
TCMAX = max(c[1] for c in CHUNKS)


def _subtiles(tcc):
    """Split a chunk into <=TS sub-tiles."""
    out, o = [], 0
    while o < tcc:
        s = min(TS, tcc - o)
        out.append((o, s))
        o += s
    return out


def build_nc(B_loc, cw, C_=C, T_=T):
    assert B_loc == B_LOC
    d = 1.0 - 1.0 / cw
    assert sum(c[1] for c in CHUNKS) == T_

    nc = bacc.Bacc(trn_type="TRN2")
    # x/out DRAM layout: flat [p, b*T*NCB] with per-(b, chunk) contiguous
    # blocks laid [cb, t] (see make_in_maps).
    xin = nc.declare_dram_parameter("x", [P, B_loc * NCB * T_], F16,
                                    isOutput=False)
    w1e = nc.declare_dram_parameter("w1e", [P, NCB * CSQ], F16, isOutput=False)
    w2d = nc.declare_dram_parameter("w2d", [M2, C_], F16, isOutput=False)
    b1d = nc.declare_dram_parameter("b1d", [M2, 1], F32, isOutput=False)
    b2e = nc.declare_dram_parameter("b2e", [P, NCB], F32, isOutput=False)
    out = nc.declare_dram_parameter("out", [P, B_loc * NCB * T_], F16,
                                    isOutput=True)

    def dslice(dram, b, ci):
        t0, tcc = CHUNKS[ci]
        off = b * (NCB * T_) + t0 * NCB
        return dram[:, off:off + NCB * tcc]

    with ExitStack() as ctx:
        tc = ctx.enter_context(tile.TileContext(nc))
        const = ctx.enter_context(tc.tile_pool(name="const", bufs=1))
        # All chunks live in SBUF at once: loads all issue up front.
        xpool = ctx.enter_context(
            tc.tile_pool(name="xp", bufs=B_loc * NTH))
        opool = ctx.enter_context(tc.tile_pool(name="op", bufs=3))
        gpool = ctx.enter_context(tc.tile_pool(name="gp", bufs=3))
        upool = ctx.enter_context(tc.tile_pool(name="up", bufs=3))
        hpool = ctx.enter_context(tc.tile_pool(name="hp", bufs=3))
        cpool = ctx.enter_context(tc.tile_pool(name="cp", bufs=2))
        php = ctx.enter_context(tc.tile_pool(name="php", bufs=2, space="PSUM"))
        pgp = ctx.enter_context(tc.tile_pool(name="pgp", bufs=2, space="PSUM"))

        # Consts ride the Scalar HWDGE ring so the Sync ring starts on x
        # immediately (they finish long before the first sigmoid).
        w1_t = const.tile([P, NCB * CSQ], F16, tag="w1e")
        nc.scalar.dma_start(w1_t[:], w1e[:])
        w2_t = const.tile([M2, C_], F16, tag="w2d")
        nc.scalar.dma_start(w2_t[:], w2d[:])
        b1_t = const.tile([M2, 1], F32, tag="b1d")
        nc.scalar.dma_start(b1_t[:], b1d[:])
        b2_t = const.tile([P, NCB], F32, tag="b2e")
        nc.scalar.dma_start(b2_t[:], b2e[:])
        dconst = const.tile([M2, TS], F32, tag="dconst")
        nc.vector.memset(dconst[:], d)

        # Issue every load immediately; the Sync ring drains them at HBM
        # rate with nothing in the way.
        xts = {}
        for ci in range(NTH):
            _, tcc = CHUNKS[ci]
            for b in range(B_loc):
                xt = xpool.tile([P, NCB * TCMAX], F16, tag="x",
                                name=f"x{b}_{ci}")
                xw3 = xt[:].rearrange("p (cb t) -> p cb t", cb=NCB, t=TCMAX)
                xv3 = dslice(xin, b, ci).rearrange(
                    "p (cb t) -> p cb t", cb=NCB, t=tcc)
                if ci == 0:
                    # First chunk arrives in sub-tile halves so the scan
                    # spine (and with it the first sigmoid) starts ~1.5us
                    # sooner.
                    nc.sync.dma_start(xw3[:, :, 0:TS], xv3[:, :, 0:TS])
                    nc.sync.dma_start(xw3[:, :, TS:tcc], xv3[:, :, TS:tcc])
                else:
                    nc.sync.dma_start(xw3[:, :, 0:tcc], xv3)
                xts[(b, ci)] = xt

        ph_pre = {}

        def phase1(ci):
            # mm1 for chunk ci, both batches stacked into one PSUM tile
            # (b0 -> rows 0-31, b1 -> rows 32-63): the two accumulation
            # chains are emission-interleaved so PE col-tiles (0,0) and
            # (0,32) co-execute.
            _, tcc = CHUNKS[ci]
            stl = _subtiles(tcc)
            xws_ = [xts[(b, ci)][:].rearrange("p (cb t) -> p cb t", cb=NCB,
                                              t=TCMAX)
                    for b in range(B_loc)]
            phs = []
            for o, s in stl:
                ph = php.tile([M2, TS], F32, tag="ph")
                for cb in range(NCB):
                    for b in range(B_loc):
                        nc.tensor.matmul(
                            ph[b * CSQ:(b + 1) * CSQ, 0:s],
                            w1_t[:, cb * CSQ:(cb + 1) * CSQ],
                            xws_[b][:, cb, o:o + s],
                            start=(cb == 0), stop=(cb == NCB - 1))
                phs.append(ph)
            ph_pre[ci] = phs

        phase1(0)
        carry = None
        for th in range(NTH):
            if th + 1 < NTH:
                phase1(th + 1)
            t0, tcc = CHUNKS[th]
            stl = _subtiles(tcc)
            phs = ph_pre.pop(th)
            # Phase 2: one scan per sub-tile + one fused relu per chunk,
            # covering BOTH batches (stacked rows).
            ut = upool.tile([M2, TCMAX], F32, tag="u")
            for k, (o, s) in enumerate(stl):
                if th == 0 and k == 0:
                    # u_0 = cw * p_0 makes y[0] = x[0] exact.
                    init = cpool.tile([M2, 1], F32, tag="c")
                    nc.vector.tensor_scalar_mul(
                        init[:], phs[k][:, 0:1], float(cw))
                    init_ap = init[:]
                else:
                    init_ap = carry
                nc.vector.tensor_tensor_scan(
                    ut[:, o:o + s], dconst[:, 0:s],
                    phs[k][:, 0:s], init_ap,
                    mybir.AluOpType.mult, mybir.AluOpType.add)
                carry = ut[:, o + s - 1:o + s]
            # Fused (u + b1) -> max(., 0) on the DVE keeps ACT free for
            # sigmoids.  Bias row layout matches the stacked batches.
            ht = hpool.tile([M2, TCMAX], F16, tag="h")
            nc.vector.tensor_scalar(
                ht[:, 0:tcc], ut[:, 0:tcc], b1_t[:], 0.0,
                mybir.AluOpType.add, mybir.AluOpType.max)
            # Phase 3: mm2 + sigmoid per (b, cb); all sub-tiles of the
            # chunk land in one PSUM tile -> one sigmoid each, b2 riding
            # the ACTIVATE bias.  b0/b1 interleaved (PE row-tiles T0/T4).
            gts = [gpool.tile([P, NCB * TCMAX], F16, tag="g", name=f"g{b}")
                   for b in range(B_loc)]
            gws = [g[:].rearrange("p (cb t) -> p cb t", cb=NCB, t=TCMAX)
                   for g in gts]
            for cb in range(NCB):
                pgs = [pgp.tile([P, TCMAX], F32, tag="pg", name=f"pg{b}")
                       for b in range(B_loc)]
                for o, s in stl:
                    for b in range(B_loc):
                        nc.tensor.matmul(
                            pgs[b][:, o:o + s],
                            w2_t[b * CSQ:(b + 1) * CSQ, cb * P:(cb + 1) * P],
                            ht[b * CSQ:(b + 1) * CSQ, o:o + s],
                            start=True, stop=True)
                for b in range(B_loc):
                    nc.scalar.activation(
                        gws[b][:, cb, 0:tcc], pgs[b][:, 0:tcc],
                        mybir.ActivationFunctionType.Sigmoid,
                        bias=b2_t[:, cb:cb + 1])
            # Phase 4: gate multiply + store per (batch, cb): each store
            # streams as soon as its cb's sigmoid lands, keeping the tail
            # (after the final sigmoid) to one small mul+store.  The
            # multiply writes a fresh fp16 tile (all-16-bit, packed DVE
            # rate); stores go out on the GpSimd SWDGE ring so they can
            # never head-of-line block the Sync ring's loads.
            for b in range(B_loc):
                ot = opool.tile([P, NCB * TCMAX], F16, tag="o", name=f"o{b}")
                ow = ot[:].rearrange("p (cb t) -> p cb t", cb=NCB, t=TCMAX)
                xw = xts.pop((b, th))[:].rearrange(
                    "p (cb t) -> p cb t", cb=NCB, t=TCMAX)
                dv = dslice(out, b, th).rearrange(
                    "p (cb t) -> p cb t", cb=NCB, t=tcc)
                for cb in range(NCB):
                    nc.vector.tensor_mul(
                        ow[:, cb, 0:tcc], xw[:, cb, 0:tcc],
                        gws[b][:, cb, 0:tcc])
                    nc.gpsimd.dma_start(dv[:, cb, :], ow[:, cb, 0:tcc])
    nc.compile()
    return nc


def make_in_maps(x, w1, b1, w2, b2, cw, n_cores=N_CORES):
    """Host-side shard + weight prep. Returns per-core input maps."""
    a = 1.0 / cw
    C_ = w2.shape[0]
    b_loc = x.shape[0] // n_cores

    w1sT = (np.asarray(w1) * a).T.astype(np.float32)      # [C, CSQ]
    w1e = np.empty((P, NCB * CSQ), dtype=np.float16)
    for cb in range(NCB):
        w1e[:, cb * CSQ:(cb + 1) * CSQ] = w1sT[cb * P:(cb + 1) * P, :]

    w2d = np.empty((M2, C_), dtype=np.float16)
    for b in range(b_loc):
        w2d[b * CSQ:(b + 1) * CSQ, :] = np.asarray(w2).T

    b1d = np.empty((M2, 1), dtype=np.float32)
    for b in range(b_loc):
        b1d[b * CSQ:(b + 1) * CSQ, 0] = np.asarray(b1)

    b2e = np.asarray(b2).astype(np.float32).reshape(NCB, P).T.copy()

    # [B, C, T] -> per-core flat [P, b*(chunk-major [cb, t])] fp16.
    x16 = np.asarray(x).astype(np.float16)
    x16 = x16.reshape(n_cores, b_loc, NCB, P, T)
    xf = np.empty((n_cores, P, b_loc * NCB * T), dtype=np.float16)
    for b in range(b_loc):
        for (t0, tcc) in CHUNKS:
            off = b * (NCB * T) + t0 * NCB
            blk = x16[:, b, :, :, t0:t0 + tcc]        # [core, cb, p, t]
            xf[:, :, off:off + NCB * tcc] = (
                blk.transpose(0, 2, 1, 3).reshape(n_cores, P, NCB * tcc))

    return [
        {"x": xf[i], "w1e": w1e, "w2d": w2d, "b1d": b1d, "b2e": b2e}
        for i in range(n_cores)
    ]


def unshard_out(results, n_cores=N_CORES, b_loc=B_LOC):
    """Per-core flat fp16 -> full [B, C, T] fp32."""
    o = np.stack([r["out"] for r in results], axis=0)  # [core, P, b*NCB*T]
    full = np.empty((n_cores, b_loc, NCB, P, T), dtype=np.float32)
    for b in range(b_loc):
        for (t0, tcc) in CHUNKS:
            off = b * (NCB * T) + t0 * NCB
            blk = o[:, :, off:off + NCB * tcc].reshape(n_cores, P, NCB, tcc)
            full[:, b, :, :, t0:t0 + tcc] = blk.transpose(0, 2, 1, 3)
    return full.reshape(B, C, T)


_NC_CACHE = {}


def kernel(x, w1, b1, w2, b2, context_window):
    cw = int(context_window)
    x = np.asarray(x)
    key = (cw, x.shape)
    if key not in _NC_CACHE:
        _NC_CACHE[key] = build_nc(x.shape[0] // N_CORES, cw)
    nc = _NC_CACHE[key]
    in_maps = make_in_maps(
        np.asarray(x), np.asarray(w1), np.asarray(b1),
        np.asarray(w2), np.asarray(b2), cw)
    res = run_bass_kernel_spmd(nc, in_maps, core_ids=list(range(N_CORES)))
    return unshard_out(res.results)
